# revision 12
# baseline (speedup 1.0000x reference)
"""Trainium2 Bass kernel for the Clause-Hypergraph GNN problem.

Strategy
--------
Data-parallel over the 8 graphs (one graph per NeuronCore). All segment-sum
gather/scatter ops are reformulated as dense matmuls against per-graph
*count* matrices built host-side from the (index-only) edge/incidence lists:

    gconv:  agg = A @ u'          A[d,s] = #edges s->d     (8192x8192, fp8 exact)
    hconv:  ef  = H^T @ v         H[n,he] = #incidences    (8192x4096, fp8 exact)
            out = H @ (Binv*ef)

Hyperedges are global across the batched graph, so each core computes a
partial ef ([4096,128]) and an AllReduce sums it across the 8 cores.

Layout: GNN state h is kept embed-major ("eT", [128=embed, 8192=nodes]) in
SBUF fp16; matmul accumulation is fp32 in PSUM; the sparse count matrices
stream through the PE as the fp8 moving operand (exact for counts <= 16).
"""

import numpy as np
import ml_dtypes

import concourse.bass as bass
import concourse.mybir as mybir
import concourse.tile as tile
from concourse import bacc
from concourse.bass_utils import run_bass_kernel_spmd
from concourse.masks import make_identity

F32 = mybir.dt.float32
F16 = mybir.dt.float16
F8 = mybir.dt.float8e4

NP_F16 = np.float16
NP_F8 = ml_dtypes.float8_e4m3

AF = mybir.ActivationFunctionType
ALU = mybir.AluOpType
AXX = mybir.AxisListType.X


class Cfg:
    def __init__(self, BS=8, NP=8192, NHE=4096, ENC=512, L=512, EMBED=128,
                 OUTH=128, n_cores=8):
        assert EMBED == 128
        self.BS, self.NP, self.NHE, self.ENC, self.L = BS, NP, NHE, ENC, L
        self.EMBED, self.OUTH, self.n_cores = EMBED, OUTH, n_cores
        self.nt = NP // 128            # node tiles
        self.het = NHE // 128          # hyperedge tiles
        self.enct = ENC // 128
        self.lt = L // 128
        self.DCW = min(512, NP)        # node-chunk width (moving free dim)
        self.ndch = NP // self.DCW
        self.HECW = min(512, NHE)
        self.nhech = NHE // self.HECW
        self.SBLK = min(16, self.nt)   # node tiles per streaming DMA
        self.HBLK = min(16, self.het)  # hyperedge tiles per streaming DMA

    def key(self):
        return (self.BS, self.NP, self.NHE, self.ENC, self.L, self.OUTH,
                self.n_cores)


def input_specs(c: Cfg):
    """(name, per-core shape, mybir dtype, numpy dtype) for all device inputs."""
    return [
        ("nfrT", [128, c.NP], F16, NP_F16),
        ("xT", [c.ENC, c.L], F32, np.float32),
        ("AT", [c.ndch, c.nt // c.SBLK, 128, c.SBLK, c.DCW], F8, NP_F8),
        ("HN", [c.nhech, c.nt // c.SBLK, 128, c.SBLK, c.HECW], F8, NP_F8),
        ("HET", [c.ndch, c.het // c.HBLK, 128, c.HBLK, c.DCW], F8, NP_F8),
        ("rdo", [128, c.nt], F32, np.float32),
        ("rdi", [128, c.nt], F32, np.float32),
        ("dinv_row", [1, c.NP], F16, NP_F16),
        ("binv", [128, c.het], F32, np.float32),
        ("bgW1", [1, 128], F16, NP_F16),
        ("bgW2", [1, 128], F16, NP_F16),
        ("Wg1", [128, 128], F16, NP_F16),
        ("Wg2", [128, 128], F16, NP_F16),
        ("Wh1", [128, 128], F16, NP_F16),
        ("Wh2", [128, 128], F16, NP_F16),
        ("Wm", [c.ENC, 128], F32, np.float32),
        ("Wm2", [128, 128], F16, NP_F16),
        ("Ws", [c.ENC + 128, c.OUTH], F32, np.float32),
        ("Wt", [c.ENC + 128, c.OUTH], F32, np.float32),
        ("bh1", [128, 1], F32, np.float32),
        ("bh2", [128, 1], F32, np.float32),
        ("bm", [128, 1], F32, np.float32),
        ("bm2", [128, 1], F32, np.float32),
        ("bs", [c.OUTH, 1], F32, np.float32),
        ("bt", [c.OUTH, 1], F32, np.float32),
    ]


def build_program(c: Cfg, reps: int = 1):
    nc = bacc.Bacc("TRN2", target_bir_lowering=False, debug=False,
                   num_devices=c.n_cores)

    d = {}
    for name, shape, dt, _ in input_specs(c):
        d[name] = nc.dram_tensor(name, shape, dt, kind="ExternalInput").ap()
    out_dram = nc.dram_tensor("out", [c.L, 2 * c.OUTH], F32,
                              kind="ExternalOutput").ap()

    with tile.TileContext(nc) as tc:
        for _ in range(reps):
            _emit(tc, c, d, out_dram)
    nc.compile()
    return nc


def _emit(tc, c: Cfg, d, out_dram):
    nc = tc.nc
    nt, het, enct, lt = c.nt, c.het, c.enct, c.lt
    DCW, ndch, HECW, nhech = c.DCW, c.ndch, c.HECW, c.nhech

    const = tc.alloc_tile_pool(name="const", bufs=1)
    state = tc.alloc_tile_pool(name="state", bufs=1)
    psum = tc.alloc_tile_pool(name="psum", bufs=3, space="PSUM")
    psum_s = tc.alloc_tile_pool(name="psum_s", bufs=2, space="PSUM")

    def load_const(name, shape, dtype, src_ap):
        t = const.tile(shape, dtype, tag=name)
        nc.sync.dma_start(t[:], src_ap)
        return t

    # --- constants ---
    Wg = [load_const("Wg1", [128, 128], F16, d["Wg1"][:]),
          load_const("Wg2", [128, 128], F16, d["Wg2"][:])]
    Wh = [load_const("Wh1", [128, 128], F16, d["Wh1"][:]),
          load_const("Wh2", [128, 128], F16, d["Wh2"][:])]
    Wm2 = load_const("Wm2", [128, 128], F16, d["Wm2"][:])
    Wm = load_const("Wm", [128, enct, 128], F32,
                    d["Wm"].rearrange("(t p) o -> p t o", p=128))
    Ws = load_const("Ws", [128, enct + 1, c.OUTH], F32,
                    d["Ws"].rearrange("(t p) o -> p t o", p=128))
    Wt = load_const("Wt", [128, enct + 1, c.OUTH], F32,
                    d["Wt"].rearrange("(t p) o -> p t o", p=128))
    rdo = load_const("rdo", [128, nt], F32, d["rdo"][:])
    rdi = load_const("rdi", [128, nt], F32, d["rdi"][:])
    binv = load_const("binv", [128, het], F32, d["binv"][:])
    bh = [load_const("bh1", [128, 1], F32, d["bh1"][:]),
          load_const("bh2", [128, 1], F32, d["bh2"][:])]
    bm = load_const("bm", [128, 1], F32, d["bm"][:])
    bm2 = load_const("bm2", [128, 1], F32, d["bm2"][:])
    bs = load_const("bs", [c.OUTH, 1], F32, d["bs"][:])
    bt = load_const("bt", [c.OUTH, 1], F32, d["bt"][:])
    xT = load_const("xT", [128, enct, c.L], F32,
                    d["xT"].rearrange("(t p) l -> p t l", p=128))

    ident = const.tile([128, 128], F32, tag="ident")
    make_identity(nc, ident[:])
    ident16 = const.tile([128, 128], F16, tag="ident16")
    make_identity(nc, ident16[:])
    ones_row = const.tile([1, 128], F16, tag="ones_row")
    nc.vector.memset(ones_row[:], 1.0)

    # final GNN state (survives into the attention phase)
    g = state.tile([128, c.NP], F16, tag="g")

    # --- GNN phase ---
    dram = tc.alloc_tile_pool(name="dram", bufs=2, space="DRAM")
    work = tc.alloc_tile_pool(name="work", bufs=1)
    mats = tc.alloc_tile_pool(name="mats", bufs=3)

    nfrT = work.tile([128, c.NP], F16, tag="nfrT")
    nc.sync.dma_start(nfrT[:], d["nfrT"][:])

    # broadcast Dinv row across all 128 partitions via K=1 ones-matmul
    dinv_row = work.tile([1, c.NP], F16, tag="dinv_row")
    nc.sync.dma_start(dinv_row[:], d["dinv_row"][:])
    dinv_bc = work.tile([128, c.NP], F16, tag="dinv_bc")
    for ci in range(ndch):
        ps = psum.tile([128, DCW], F32, tag="ps_main")
        nc.tensor.matmul(ps[:], ones_row[:], dinv_row[:, ci * DCW:(ci + 1) * DCW],
                         start=True, stop=True)
        nc.vector.tensor_copy(dinv_bc[:, ci * DCW:(ci + 1) * DCW], ps[:])

    bgW_bc = []
    for li in range(2):
        row = work.tile([1, 128], F16, tag=f"bgW_row{li}")
        nc.sync.dma_start(row[:], d[f"bgW{li + 1}"][:])
        t = work.tile([128, 128], F16, tag=f"bgW_bc{li}")
        ps = psum_s.tile([128, 128], F32, tag="ps_small")
        nc.tensor.matmul(ps[:], ones_row[:], row[:], start=True, stop=True)
        nc.vector.tensor_copy(t[:], ps[:])
        bgW_bc.append(t)

    h1 = work.tile([128, c.NP], F16, tag="h1")

    for li in range(2):
        h_in = nfrT if li == 0 else h1
        h_out = h1 if li == 0 else g

        # ---- gconv: u' = rs_dout * (h @ Wg)  (natural layout, f16) ----
        u16 = work.tile([128, nt, 128], F16, tag="stat16")
        for t in range(nt):
            ps = psum_s.tile([128, 128], F32, tag="ps_small")
            nc.tensor.matmul(ps[:], h_in[:, t * 128:(t + 1) * 128], Wg[li][:],
                             start=True, stop=True)
            nc.vector.tensor_scalar_mul(u16[:, t, :], ps[:], rdo[:, t:t + 1])

        # ---- aggT = u'^T @ A^T : stream AT (fp8) as moving operand ----
        aggT = work.tile([128, c.NP], F16, tag="aggT")
        for ci in range(ndch):
            ps = psum.tile([128, DCW], F32, tag="ps_main")
            for sb in range(nt // c.SBLK):
                mt = mats.tile([128, c.SBLK, DCW], F8, tag="mat")
                nc.sync.dma_start(mt[:], d["AT"][ci, sb])
                for j in range(c.SBLK):
                    s = sb * c.SBLK + j
                    nc.tensor.matmul(ps[:], u16[:, s, :], mt[:, j, :],
                                     start=(s == 0), stop=(s == nt - 1))
            nc.vector.tensor_copy(aggT[:, ci * DCW:(ci + 1) * DCW], ps[:])

        # ---- hconv in: v = rs_din * (agg @ Wh) + (bg @ Wh) ----
        v16 = work.tile([128, nt, 128], F16, tag="stat16")
        for t in range(nt):
            ps = psum_s.tile([128, 128], F32, tag="ps_small")
            nc.tensor.matmul(ps[:], aggT[:, t * 128:(t + 1) * 128], Wh[li][:],
                             start=True, stop=True)
            nc.vector.scalar_tensor_tensor(
                v16[:, t, :], ps[:], rdi[:, t:t + 1], bgW_bc[li][:],
                op0=ALU.mult, op1=ALU.add)

        # ---- ef_partial^T = v^T @ HN (fp8 moving) ----
        efT = work.tile([128, c.NHE], F16, tag="efT")
        for hc in range(nhech):
            ps = psum.tile([128, HECW], F32, tag="ps_main")
            for sb in range(nt // c.SBLK):
                mt = mats.tile([128, c.SBLK, HECW], F8, tag="mat")
                nc.sync.dma_start(mt[:], d["HN"][hc, sb])
                for j in range(c.SBLK):
                    s = sb * c.SBLK + j
                    nc.tensor.matmul(ps[:], v16[:, s, :], mt[:, j, :],
                                     start=(s == 0), stop=(s == nt - 1))
            nc.vector.tensor_copy(efT[:, hc * HECW:(hc + 1) * HECW], ps[:])

        # ---- AllReduce ef over all cores (hyperedges are global) ----
        efts = work.tile([128, c.NHE], F16, tag="efts")
        if c.n_cores > 1:
            cc_in = dram.tile([128, c.NHE], F16)
            cc_out = dram.tile([128, c.NHE], F16)
            nc.sync.dma_start(cc_in[:], efT[:])
            nc.gpsimd.collective_compute(
                "AllReduce", ALU.add,
                replica_groups=[list(range(c.n_cores))],
                ins=[cc_in.opt()], outs=[cc_out.opt()])
            nc.sync.dma_start(efts[:], cc_out[:])
        else:
            nc.vector.tensor_copy(efts[:], efT[:])

        # ---- ef natural ([he,e]) via PE transpose, * Binv on evac ----
        efnat = work.tile([128, het, 128], F16, tag="efnat")
        for t in range(het):
            pst = psum_s.tile([128, 128], F16, tag="ps_tr")
            nc.tensor.transpose(pst[:], efts[:, t * 128:(t + 1) * 128],
                                ident16[:])
            nc.vector.tensor_scalar_mul(efnat[:, t, :], pst[:],
                                        binv[:, t:t + 1])

        # ---- out^T = ef'^T @ HET; h = relu(Dinv*out + bh) + nfr ----
        for ci in range(ndch):
            ps = psum.tile([128, DCW], F32, tag="ps_main")
            for hb in range(het // c.HBLK):
                mt = mats.tile([128, c.HBLK, DCW], F8, tag="mat")
                nc.sync.dma_start(mt[:], d["HET"][ci, hb])
                for j in range(c.HBLK):
                    t = hb * c.HBLK + j
                    nc.tensor.matmul(ps[:], efnat[:, t, :], mt[:, j, :],
                                     start=(t == 0), stop=(t == het - 1))
            sl = slice(ci * DCW, (ci + 1) * DCW)
            tmp = work.tile([128, DCW], F32, tag="evac")
            nc.vector.tensor_tensor(tmp[:], ps[:], dinv_bc[:, sl], ALU.mult)
            nc.scalar.activation(tmp[:], tmp[:], AF.Relu, bias=bh[li][:])
            nc.vector.tensor_tensor(h_out[:, sl], tmp[:], nfrT[:, sl], ALU.add)

    for p in (mats, work):
        p.release()

    # --- attention phase ---
    att = tc.alloc_tile_pool(name="att", bufs=1)
    att2 = tc.alloc_tile_pool(name="att2", bufs=8)

    # kT = relu(Wm2^T @ g + bm2)   [e, n] f16
    kT = att.tile([128, c.NP], F16, tag="kT")
    for ci in range(ndch):
        ps = psum.tile([128, DCW], F32, tag="ps_main")
        nc.tensor.matmul(ps[:], Wm2[:], g[:, ci * DCW:(ci + 1) * DCW],
                         start=True, stop=True)
        nc.scalar.activation(kT[:, ci * DCW:(ci + 1) * DCW], ps[:], AF.Relu,
                             bias=bm2[:])

    # qT = relu(Wm^T @ xT + bm)    [e, l] f16
    qT = att.tile([128, c.L], F16, tag="qT")
    ps = psum.tile([128, c.L], F32, tag="ps_main")
    for et in range(enct):
        nc.tensor.matmul(ps[:], Wm[:, et, :], xT[:, et, :],
                         start=(et == 0), stop=(et == enct - 1))
    nc.scalar.activation(qT[:], ps[:], AF.Relu, bias=bm[:])

    # g natural tiles for the P@g matmul
    gnat = att.tile([128, nt, 128], F16, tag="gnat")
    for t in range(nt):
        nc.sync.dma_start_transpose(gnat[:, t, :], g[:, t * 128:(t + 1) * 128])

    # C = q @ k^T  -> softmax rows (over nodes). Row max is collected for
    # free during the psum->SBUF evacuation (tensor_scalar accum_out).
    C16 = att.tile([128, lt, c.NP], F16, tag="C16")
    cmax = att.tile([128, lt, ndch], F32, tag="cmax")
    for l in range(lt):
        for ci in range(ndch):
            ps = psum.tile([128, DCW], F32, tag="ps_main")
            nc.tensor.matmul(ps[:], qT[:, l * 128:(l + 1) * 128],
                             kT[:, ci * DCW:(ci + 1) * DCW],
                             start=True, stop=True)
            nc.vector.tensor_scalar(C16[:, l, ci * DCW:(ci + 1) * DCW], ps[:],
                                    0.0, None, op0=ALU.add, op1=ALU.max,
                                    accum_out=cmax[:, l, ci:ci + 1])
    for l in range(lt):
        negmax = att2.tile([128, 1], F32, tag="negmax")
        rsum = att2.tile([128, 1], F32, tag="rsum")
        rs = att2.tile([128, 1], F32, tag="rs")
        nc.vector.tensor_reduce(negmax[:], cmax[:, l, :], axis=AXX, op=ALU.max,
                                negate=True)
        nc.vector.memset(rsum[:], 0.0)
        nc.scalar.activation(C16[:, l, :], C16[:, l, :], AF.Exp,
                             bias=negmax[:], accum_out=rsum[:])
        nc.vector.reciprocal(rs[:], rsum[:])
        nc.vector.tensor_scalar_mul(C16[:, l, :], C16[:, l, :], rs[:])

    # H^T = g^T @ P^T   [e, l] f32
    ps_h = psum.tile([128, c.L], F32, tag="ps_main")
    for t in range(nt):
        pt = att2.tile([128, lt, 128], F16, tag="pt")
        for l in range(lt):
            nc.sync.dma_start_transpose(pt[:, l, :],
                                        C16[:, l, t * 128:(t + 1) * 128])
        nc.tensor.matmul(ps_h[:], gnat[:, t, :], pt[:], start=(t == 0),
                         stop=(t == nt - 1))
    HT = att.tile([128, c.L], F32, tag="HT")
    nc.vector.tensor_copy(HT[:], ps_h[:])

    # G1/G2 = sigmoid/tanh([x, H] @ W + b), then transpose to [l, o] and store
    onat = att.tile([128, lt, 2 * c.OUTH], F32, tag="onat")
    for (W_sb, b_sb, fn, half) in ((Ws, bs, AF.Sigmoid, 0), (Wt, bt, AF.Tanh, 1)):
        ps = psum.tile([c.OUTH, c.L], F32, tag="ps_main")
        for et in range(enct):
            nc.tensor.matmul(ps[:], W_sb[:, et, :], xT[:, et, :],
                             start=(et == 0), stop=False)
        nc.tensor.matmul(ps[:], W_sb[:, enct, :], HT[:], start=False, stop=True)
        GT = att.tile([c.OUTH, c.L], F32, tag=f"GT{half}")
        nc.scalar.activation(GT[:], ps[:], fn, bias=b_sb[:])
        for l in range(lt):
            pst = psum_s.tile([128, 128], F32, tag="ps_small")
            nc.tensor.transpose(pst[:], GT[:, l * 128:(l + 1) * 128], ident[:])
            nc.vector.tensor_copy(
                onat[:, l, half * c.OUTH:(half + 1) * c.OUTH], pst[:, :c.OUTH])
    for l in range(lt):
        nc.sync.dma_start(out_dram[l * 128:(l + 1) * 128, :], onat[:, l, :])

    for p in (att2, att, dram, psum_s, psum, state, const):
        p.release()


# --------------------------------------------------------------------------
# Host side
# --------------------------------------------------------------------------

def preprocess(inputs, c: Cfg):
    """Build per-core input maps from the full problem inputs (host, numpy).

    Only index-derived quantities (counts/degrees) and relayouts happen here;
    all float math on node/sequence data runs on device.
    """
    x = np.asarray(inputs["x"], np.float32)
    nf = np.asarray(inputs["node_features"], np.float32)
    ei = np.asarray(inputs["edge_index"])
    hi = np.asarray(inputs["hyperedge_index"])
    NP, NHE, BS = c.NP, c.NHE, c.BS
    N = BS * NP

    src, dst = np.asarray(ei[0], np.int64), np.asarray(ei[1], np.int64)
    pn, pe = np.asarray(hi[:, 0], np.int64), np.asarray(hi[:, 1], np.int64)

    dout = np.bincount(src, minlength=N).astype(np.float64)
    din = np.bincount(dst, minlength=N).astype(np.float64)
    rs_dout = (1.0 / np.sqrt(np.maximum(dout, 1.0))).astype(np.float32)
    rs_din = (1.0 / np.sqrt(np.maximum(din, 1.0))).astype(np.float32)
    D = np.bincount(pn, minlength=N).astype(np.float64)
    B = np.bincount(pe, minlength=NHE).astype(np.float64)
    Dinv = np.where(D > 0, 1.0 / np.maximum(D, 1), 0.0).astype(np.float32)
    Binv = np.where(B > 0, 1.0 / np.maximum(B, 1), 0.0).astype(np.float32)

    W = {k: np.asarray(inputs[k], np.float32) for k in
         ("Wg1", "bg1", "Wg2", "bg2", "Wh1", "bh1", "Wh2", "bh2",
          "Wm", "bm", "Wm2", "bm2", "Ws", "bs_", "Wt", "bt")}
    bgW1 = (W["bg1"] @ W["Wh1"]).astype(np.float32)
    bgW2 = (W["bg2"] @ W["Wh2"]).astype(np.float32)

    g_of_e = src // NP
    assert (g_of_e == dst // NP).all(), "edges must stay within graphs"
    g_of_p = pn // NP

    common = {
        "binv": np.ascontiguousarray(Binv.reshape(c.het, 128).T),
        "bgW1": bgW1.astype(NP_F16)[None, :],
        "bgW2": bgW2.astype(NP_F16)[None, :],
        "Wg1": W["Wg1"].astype(NP_F16), "Wg2": W["Wg2"].astype(NP_F16),
        "Wh1": W["Wh1"].astype(NP_F16), "Wh2": W["Wh2"].astype(NP_F16),
        "Wm": W["Wm"], "Wm2": W["Wm2"].astype(NP_F16),
        "Ws": W["Ws"], "Wt": W["Wt"],
        "bh1": W["bh1"].astype(np.float32)[:, None],
        "bh2": W["bh2"].astype(np.float32)[:, None],
        "bm": W["bm"].astype(np.float32)[:, None],
        "bm2": W["bm2"].astype(np.float32)[:, None],
        "bs": W["bs_"].astype(np.float32)[:, None],
        "bt": W["bt"].astype(np.float32)[:, None],
    }

    def regroup(M, cw, blk):
        # [R, C] -> [C//cw, R//(128*blk), 128, blk, cw]; per-partition bytes
        # of one streaming tile are contiguous in DRAM.
        R, C = M.shape
        return np.ascontiguousarray(
            M.reshape(R // (128 * blk), blk, 128, C // cw, cw)
            .transpose(3, 0, 2, 1, 4))

    in_maps = []
    for gidx in range(c.n_cores):
        lo = gidx * NP
        m = dict(common)
        m["nfrT"] = np.ascontiguousarray(nf[gidx].T).astype(NP_F16)
        m["xT"] = np.ascontiguousarray(x[gidx].T)
        m["rdo"] = np.ascontiguousarray(rs_dout[lo:lo + NP].reshape(c.nt, 128).T)
        m["rdi"] = np.ascontiguousarray(rs_din[lo:lo + NP].reshape(c.nt, 128).T)
        m["dinv_row"] = Dinv[lo:lo + NP].astype(NP_F16)[None, :]

        sel = g_of_e == gidx
        ss, dd = src[sel] - lo, dst[sel] - lo
        cnt = np.bincount(ss * NP + dd, minlength=NP * NP)
        assert cnt.max() <= 16, f"edge multiplicity {cnt.max()} > 16"
        AT = cnt.astype(NP_F8).reshape(NP, NP)          # [s, d]
        del cnt
        m["AT"] = regroup(AT, c.DCW, c.SBLK)
        del AT

        selp = g_of_p == gidx
        nn, ee = pn[selp] - lo, pe[selp]
        cnt = np.bincount(nn * NHE + ee, minlength=NP * NHE)
        assert cnt.max() <= 16, f"incidence multiplicity {cnt.max()} > 16"
        HN = cnt.astype(NP_F8).reshape(NP, NHE)          # [n, he]
        del cnt
        m["HN"] = regroup(HN, c.HECW, c.SBLK)
        m["HET"] = regroup(np.ascontiguousarray(HN.T), c.DCW, c.HBLK)
        del HN

        for name, shape, _, npdt in input_specs(c):
            arr = np.ascontiguousarray(m[name]).astype(npdt, copy=False)
            assert list(arr.shape) == list(shape), (name, arr.shape, shape)
            m[name] = arr
        in_maps.append(m)
    return in_maps


_PROGRAM_CACHE = {}


def _get_program(c: Cfg):
    k = c.key()
    if k not in _PROGRAM_CACHE:
        _PROGRAM_CACHE[k] = build_program(c)
    return _PROGRAM_CACHE[k]


def run(inputs, c: Cfg, trace=False):
    nc = _get_program(c)
    in_maps = preprocess(inputs, c)
    res = run_bass_kernel_spmd(nc, in_maps, list(range(c.n_cores)), trace=trace)
    out = np.stack([res.results[i]["out"] for i in range(c.n_cores)], axis=0)
    return out.astype(np.float32), res


def kernel(**inputs) -> np.ndarray:
    c = Cfg()
    out, _ = run(inputs, c)
    return out


# revision 15
# speedup vs baseline: 1.8558x; 1.8558x over previous
"""Trainium2 Bass kernel for the Clause-Hypergraph GNN problem.

Strategy
--------
Data-parallel over the 8 graphs (one graph per NeuronCore). All segment-sum
gather/scatter ops are reformulated as dense matmuls against per-graph
*count* matrices built host-side from the (index-only) edge/incidence lists:

    gconv:  agg = A @ u'          A[d,s] = #edges s->d     (8192x8192, fp8 exact)
    hconv:  ef  = H^T @ v         H[n,he] = #incidences    (8192x4096, fp8 exact)
            out = H @ (Binv*ef)

Hyperedges are global across the batched graph, so each core computes a
partial ef ([4096,128]) and an AllReduce sums it across the 8 cores.

Layout: GNN state h is kept embed-major ("eT", [128=embed, 8192=nodes]) in
SBUF fp16; matmul accumulation is fp32 in PSUM; the sparse count matrices
stream through the PE as the fp8 moving operand (exact for counts <= 16).
"""

import numpy as np
import ml_dtypes

import concourse.bass as bass
import concourse.mybir as mybir
import concourse.tile as tile
from concourse import bacc
from concourse.bass_utils import run_bass_kernel_spmd
from concourse.masks import make_identity

F32 = mybir.dt.float32
F16 = mybir.dt.float16
F8 = mybir.dt.float8e4

NP_F16 = np.float16
NP_F8 = ml_dtypes.float8_e4m3

AF = mybir.ActivationFunctionType
ALU = mybir.AluOpType
AXX = mybir.AxisListType.X


class Cfg:
    def __init__(self, BS=8, NP=8192, NHE=4096, ENC=512, L=512, EMBED=128,
                 OUTH=128, n_cores=8):
        assert EMBED == 128
        self.BS, self.NP, self.NHE, self.ENC, self.L = BS, NP, NHE, ENC, L
        self.EMBED, self.OUTH, self.n_cores = EMBED, OUTH, n_cores
        self.nt = NP // 128            # node tiles
        self.het = NHE // 128          # hyperedge tiles
        self.enct = ENC // 128
        self.lt = L // 128
        self.DCW = min(512, NP)        # node-chunk width (moving free dim)
        self.ndch = NP // self.DCW
        self.HECW = min(512, NHE)
        self.nhech = NHE // self.HECW
        self.SBLK = min(16, self.nt)   # node tiles per streaming DMA
        self.HBLK = min(16, self.het)  # hyperedge tiles per streaming DMA

    def key(self):
        return (self.BS, self.NP, self.NHE, self.ENC, self.L, self.OUTH,
                self.n_cores)


def input_specs(c: Cfg):
    """(name, per-core shape, mybir dtype, numpy dtype) for all device inputs."""
    return [
        ("nfrT", [128, c.NP], F16, NP_F16),
        ("xT", [c.ENC, c.L], F32, np.float32),
        ("AT", [c.ndch, c.nt // c.SBLK, 128, c.SBLK, c.DCW], F8, NP_F8),
        ("HN", [c.nhech, c.nt // c.SBLK, 128, c.SBLK, c.HECW], F8, NP_F8),
        ("HET", [c.ndch, c.het // c.HBLK, 128, c.HBLK, c.DCW], F8, NP_F8),
        ("rdo", [128, c.nt], F32, np.float32),
        ("rdi", [128, c.nt], F32, np.float32),
        ("dinv_row", [1, c.NP], F16, NP_F16),
        ("binv", [128, c.het], F32, np.float32),
        ("bgW1", [1, 128], F16, NP_F16),
        ("bgW2", [1, 128], F16, NP_F16),
        ("Wg1", [128, 128], F16, NP_F16),
        ("Wg2", [128, 128], F16, NP_F16),
        ("Wh1", [128, 128], F16, NP_F16),
        ("Wh2", [128, 128], F16, NP_F16),
        ("Wm", [c.ENC, 128], F32, np.float32),
        ("Wm2", [128, 128], F16, NP_F16),
        ("Ws", [c.ENC + 128, c.OUTH], F32, np.float32),
        ("Wt", [c.ENC + 128, c.OUTH], F32, np.float32),
        ("bh1", [128, 1], F32, np.float32),
        ("bh2", [128, 1], F32, np.float32),
        ("bm", [128, 1], F32, np.float32),
        ("bm2", [128, 1], F32, np.float32),
        ("bs", [c.OUTH, 1], F32, np.float32),
        ("bt", [c.OUTH, 1], F32, np.float32),
    ]


def build_program(c: Cfg, reps: int = 1, do_att=True, do_cc=True):
    nc = bacc.Bacc("TRN2", target_bir_lowering=False, debug=False,
                   num_devices=c.n_cores)

    d = {}
    for name, shape, dt, _ in input_specs(c):
        d[name] = nc.dram_tensor(name, shape, dt, kind="ExternalInput").ap()
    out_dram = nc.dram_tensor("out", [c.L, 2 * c.OUTH], F32,
                              kind="ExternalOutput").ap()

    with tile.TileContext(nc) as tc:
        for _ in range(reps):
            _emit(tc, c, d, out_dram, do_att=do_att, do_cc=do_cc)
    nc.compile()
    return nc


def _emit(tc, c: Cfg, d, out_dram, do_att=True, do_cc=True):
    nc = tc.nc
    nt, het, enct, lt = c.nt, c.het, c.enct, c.lt
    DCW, ndch, HECW, nhech = c.DCW, c.ndch, c.HECW, c.nhech

    const = tc.alloc_tile_pool(name="const", bufs=1)
    state = tc.alloc_tile_pool(name="state", bufs=1)
    psum = tc.alloc_tile_pool(name="psum", bufs=3, space="PSUM")
    psum_s = tc.alloc_tile_pool(name="psum_s", bufs=2, space="PSUM")

    def load_const(name, shape, dtype, src_ap):
        t = const.tile(shape, dtype, tag=name)
        nc.sync.dma_start(t[:], src_ap)
        return t

    # --- constants ---
    Wg = [load_const("Wg1", [128, 128], F16, d["Wg1"][:]),
          load_const("Wg2", [128, 128], F16, d["Wg2"][:])]
    Wh = [load_const("Wh1", [128, 128], F16, d["Wh1"][:]),
          load_const("Wh2", [128, 128], F16, d["Wh2"][:])]
    Wm2 = load_const("Wm2", [128, 128], F16, d["Wm2"][:])
    Wm = load_const("Wm", [128, enct, 128], F32,
                    d["Wm"].rearrange("(t p) o -> p t o", p=128))
    Ws = load_const("Ws", [128, enct + 1, c.OUTH], F32,
                    d["Ws"].rearrange("(t p) o -> p t o", p=128))
    Wt = load_const("Wt", [128, enct + 1, c.OUTH], F32,
                    d["Wt"].rearrange("(t p) o -> p t o", p=128))
    rdo = load_const("rdo", [128, nt], F32, d["rdo"][:])
    rdi = load_const("rdi", [128, nt], F32, d["rdi"][:])
    binv = load_const("binv", [128, het], F32, d["binv"][:])
    bh = [load_const("bh1", [128, 1], F32, d["bh1"][:]),
          load_const("bh2", [128, 1], F32, d["bh2"][:])]
    bm = load_const("bm", [128, 1], F32, d["bm"][:])
    bm2 = load_const("bm2", [128, 1], F32, d["bm2"][:])
    bs = load_const("bs", [c.OUTH, 1], F32, d["bs"][:])
    bt = load_const("bt", [c.OUTH, 1], F32, d["bt"][:])
    xT = load_const("xT", [128, enct, c.L], F32,
                    d["xT"].rearrange("(t p) l -> p t l", p=128))

    ident = const.tile([128, 128], F32, tag="ident")
    make_identity(nc, ident[:])
    ident16 = const.tile([128, 128], F16, tag="ident16")
    make_identity(nc, ident16[:])
    ones_row = const.tile([1, 128], F16, tag="ones_row")
    nc.vector.memset(ones_row[:], 1.0)

    # final GNN state (survives into the attention phase)
    g = state.tile([128, c.NP], F16, tag="g")

    # --- GNN phase ---
    dram = tc.alloc_tile_pool(name="dram", bufs=2, space="DRAM")
    work = tc.alloc_tile_pool(name="work", bufs=1)
    mats = tc.alloc_tile_pool(name="mats", bufs=4)

    nfrT = work.tile([128, c.NP], F16, tag="nfrT")
    nc.sync.dma_start(nfrT[:], d["nfrT"][:])

    # broadcast Dinv row across all 128 partitions via K=1 ones-matmul
    dinv_row = work.tile([1, c.NP], F16, tag="dinv_row")
    nc.sync.dma_start(dinv_row[:], d["dinv_row"][:])
    dinv_bc = work.tile([128, c.NP], F16, tag="dinv_bc")
    for ci in range(ndch):
        ps = psum.tile([128, DCW], F32, tag="ps_main")
        nc.tensor.matmul(ps[:], ones_row[:], dinv_row[:, ci * DCW:(ci + 1) * DCW],
                         start=True, stop=True)
        nc.vector.tensor_copy(dinv_bc[:, ci * DCW:(ci + 1) * DCW], ps[:])

    bgW_bc = []
    for li in range(2):
        row = work.tile([1, 128], F16, tag=f"bgW_row{li}")
        nc.sync.dma_start(row[:], d[f"bgW{li + 1}"][:])
        t = work.tile([128, 128], F16, tag=f"bgW_bc{li}")
        ps = psum_s.tile([128, 128], F32, tag="ps_small")
        nc.tensor.matmul(ps[:], ones_row[:], row[:], start=True, stop=True)
        nc.vector.tensor_copy(t[:], ps[:])
        bgW_bc.append(t)

    h1 = work.tile([128, c.NP], F16, tag="h1")

    for li in range(2):
        h_in = nfrT if li == 0 else h1
        h_out = h1 if li == 0 else g

        # ---- gconv: u' = rs_dout * (h @ Wg)  (natural layout, f16) ----
        # 4 transpose-matmuls share one PSUM bank; one wide DVE evac applies
        # the per-node scale via a step-0 broadcast AP.
        u16 = work.tile([128, nt, 128], F16, tag="stat16")
        TB = min(4, nt)
        for tb in range(nt // TB):
            ps = psum.tile([128, TB * 128], F32, tag="ps_main")
            for j in range(TB):
                t = tb * TB + j
                nc.tensor.matmul(ps[:, j * 128:(j + 1) * 128],
                                 h_in[:, t * 128:(t + 1) * 128], Wg[li][:],
                                 start=True, stop=True)
            psv = ps[:].rearrange("p (t e) -> p t e", t=TB)
            nc.vector.tensor_tensor(
                u16[:, tb * TB:(tb + 1) * TB, :], psv,
                rdo[:, tb * TB:(tb + 1) * TB, None].to_broadcast(
                    (128, TB, 128)), ALU.mult)

        # ---- aggT = u'^T @ A^T : stream AT (fp8) as moving operand ----
        aggT = work.tile([128, c.NP], F16, tag="aggT")
        for ci in range(ndch):
            ps = psum.tile([128, DCW], F32, tag="ps_main")
            for sb in range(nt // c.SBLK):
                mt = mats.tile([128, c.SBLK, DCW], F8, tag="mat")
                nc.sync.dma_start(mt[:], d["AT"][ci, sb])
                for j in range(c.SBLK):
                    s = sb * c.SBLK + j
                    nc.tensor.matmul(ps[:], u16[:, s, :], mt[:, j, :],
                                     start=(s == 0), stop=(s == nt - 1))
            nc.vector.tensor_copy(aggT[:, ci * DCW:(ci + 1) * DCW], ps[:])

        # ---- hconv in: v = rs_din * (agg @ Wh) + (bg @ Wh) ----
        v16 = work.tile([128, nt, 128], F16, tag="stat16")
        for tb in range(nt // TB):
            ps = psum.tile([128, TB * 128], F32, tag="ps_main")
            for j in range(TB):
                t = tb * TB + j
                nc.tensor.matmul(ps[:, j * 128:(j + 1) * 128],
                                 aggT[:, t * 128:(t + 1) * 128], Wh[li][:],
                                 start=True, stop=True)
            psv = ps[:].rearrange("p (t e) -> p t e", t=TB)
            vsl = v16[:, tb * TB:(tb + 1) * TB, :]
            nc.vector.tensor_tensor(
                vsl, psv,
                rdi[:, tb * TB:(tb + 1) * TB, None].to_broadcast(
                    (128, TB, 128)), ALU.mult)
            nc.vector.tensor_tensor(
                vsl, vsl,
                bgW_bc[li][:, None, :].to_broadcast((128, TB, 128)), ALU.add)

        # ---- ef_partial^T = v^T @ HN (fp8 moving) ----
        efT = work.tile([128, c.NHE], F16, tag="efT")
        for hc in range(nhech):
            ps = psum.tile([128, HECW], F32, tag="ps_main")
            for sb in range(nt // c.SBLK):
                mt = mats.tile([128, c.SBLK, HECW], F8, tag="mat")
                nc.sync.dma_start(mt[:], d["HN"][hc, sb])
                for j in range(c.SBLK):
                    s = sb * c.SBLK + j
                    nc.tensor.matmul(ps[:], v16[:, s, :], mt[:, j, :],
                                     start=(s == 0), stop=(s == nt - 1))
            nc.vector.tensor_copy(efT[:, hc * HECW:(hc + 1) * HECW], ps[:])

        # ---- AllReduce ef over all cores (hyperedges are global) ----
        efts = work.tile([128, c.NHE], F16, tag="efts")
        if c.n_cores > 1 and do_cc:
            cc_in = dram.tile([128, c.NHE], F16)
            cc_out = dram.tile([128, c.NHE], F16)
            nc.sync.dma_start(cc_in[:], efT[:])
            nc.gpsimd.collective_compute(
                "AllReduce", ALU.add,
                replica_groups=[list(range(c.n_cores))],
                ins=[cc_in.opt()], outs=[cc_out.opt()])
            nc.sync.dma_start(efts[:], cc_out[:])
        else:
            nc.vector.tensor_copy(efts[:], efT[:])

        # ---- ef natural ([he,e]) via PE transpose, * Binv on evac ----
        efnat = work.tile([128, het, 128], F16, tag="efnat")
        for t in range(het):
            pst = psum_s.tile([128, 128], F16, tag="ps_tr")
            nc.tensor.transpose(pst[:], efts[:, t * 128:(t + 1) * 128],
                                ident16[:])
            nc.vector.tensor_scalar_mul(efnat[:, t, :], pst[:],
                                        binv[:, t:t + 1])

        # ---- out^T = ef'^T @ HET; h = relu(Dinv*out + bh) + nfr ----
        for ci in range(ndch):
            ps = psum.tile([128, DCW], F32, tag="ps_main")
            for hb in range(het // c.HBLK):
                mt = mats.tile([128, c.HBLK, DCW], F8, tag="mat")
                nc.sync.dma_start(mt[:], d["HET"][ci, hb])
                for j in range(c.HBLK):
                    t = hb * c.HBLK + j
                    nc.tensor.matmul(ps[:], efnat[:, t, :], mt[:, j, :],
                                     start=(t == 0), stop=(t == het - 1))
            sl = slice(ci * DCW, (ci + 1) * DCW)
            tmp = work.tile([128, DCW], F32, tag="evac")
            nc.vector.tensor_tensor(tmp[:], ps[:], dinv_bc[:, sl], ALU.mult)
            nc.scalar.activation(tmp[:], tmp[:], AF.Relu, bias=bh[li][:])
            nc.vector.tensor_tensor(h_out[:, sl], tmp[:], nfrT[:, sl], ALU.add)

    for p in (mats, work):
        p.release()

    # --- attention phase ---
    att = tc.alloc_tile_pool(name="att", bufs=1)
    att2 = tc.alloc_tile_pool(name="att2", bufs=8)
    if not do_att:
        onat0 = att.tile([128, 2 * c.OUTH], F32, tag="onat0")
        nc.vector.tensor_copy(onat0[:], g[:, None, :2 * c.OUTH])
        for l in range(lt):
            nc.sync.dma_start(out_dram[l * 128:(l + 1) * 128, :], onat0[:])
        for p in (att2, att, dram, psum_s, psum, state, const):
            p.release()
        return

    # kT = relu(Wm2^T @ g + bm2)   [e, n] f16
    kT = att.tile([128, c.NP], F16, tag="kT")
    for ci in range(ndch):
        ps = psum.tile([128, DCW], F32, tag="ps_main")
        nc.tensor.matmul(ps[:], Wm2[:], g[:, ci * DCW:(ci + 1) * DCW],
                         start=True, stop=True)
        nc.scalar.activation(kT[:, ci * DCW:(ci + 1) * DCW], ps[:], AF.Relu,
                             bias=bm2[:])

    # qT = relu(Wm^T @ xT + bm)    [e, l] f16
    qT = att.tile([128, c.L], F16, tag="qT")
    ps = psum.tile([128, c.L], F32, tag="ps_main")
    for et in range(enct):
        nc.tensor.matmul(ps[:], Wm[:, et, :], xT[:, et, :],
                         start=(et == 0), stop=(et == enct - 1))
    nc.scalar.activation(qT[:], ps[:], AF.Relu, bias=bm[:])

    # g natural tiles for the P@g matmul
    gnat = att.tile([128, nt, 128], F16, tag="gnat")
    for t in range(nt):
        nc.sync.dma_start_transpose(gnat[:, t, :], g[:, t * 128:(t + 1) * 128])

    # C = q @ k^T  -> softmax rows (over nodes). Row max is collected for
    # free during the psum->SBUF evacuation (tensor_scalar accum_out).
    C16 = att.tile([128, lt, c.NP], F16, tag="C16")
    cmax = att.tile([128, lt, ndch], F32, tag="cmax")
    for l in range(lt):
        for ci in range(ndch):
            ps = psum.tile([128, DCW], F32, tag="ps_main")
            nc.tensor.matmul(ps[:], qT[:, l * 128:(l + 1) * 128],
                             kT[:, ci * DCW:(ci + 1) * DCW],
                             start=True, stop=True)
            nc.vector.tensor_scalar(C16[:, l, ci * DCW:(ci + 1) * DCW], ps[:],
                                    0.0, None, op0=ALU.add, op1=ALU.max,
                                    accum_out=cmax[:, l, ci:ci + 1])
    for l in range(lt):
        negmax = att2.tile([128, 1], F32, tag="negmax")
        rsum = att2.tile([128, 1], F32, tag="rsum")
        rs = att2.tile([128, 1], F32, tag="rs")
        nc.vector.tensor_reduce(negmax[:], cmax[:, l, :], axis=AXX, op=ALU.max,
                                negate=True)
        nc.vector.memset(rsum[:], 0.0)
        nc.scalar.activation(C16[:, l, :], C16[:, l, :], AF.Exp,
                             bias=negmax[:], accum_out=rsum[:])
        nc.vector.reciprocal(rs[:], rsum[:])
        nc.vector.tensor_scalar_mul(C16[:, l, :], C16[:, l, :], rs[:])

    # H^T = g^T @ P^T   [e, l] f32
    ps_h = psum.tile([128, c.L], F32, tag="ps_main")
    for t in range(nt):
        pt = att2.tile([128, lt, 128], F16, tag="pt")
        for l in range(lt):
            nc.sync.dma_start_transpose(pt[:, l, :],
                                        C16[:, l, t * 128:(t + 1) * 128])
        nc.tensor.matmul(ps_h[:], gnat[:, t, :], pt[:], start=(t == 0),
                         stop=(t == nt - 1))
    HT = att.tile([128, c.L], F32, tag="HT")
    nc.vector.tensor_copy(HT[:], ps_h[:])

    # G1/G2 = sigmoid/tanh([x, H] @ W + b), then transpose to [l, o] and store
    onat = att.tile([128, lt, 2 * c.OUTH], F32, tag="onat")
    for (W_sb, b_sb, fn, half) in ((Ws, bs, AF.Sigmoid, 0), (Wt, bt, AF.Tanh, 1)):
        ps = psum.tile([c.OUTH, c.L], F32, tag="ps_main")
        for et in range(enct):
            nc.tensor.matmul(ps[:], W_sb[:, et, :], xT[:, et, :],
                             start=(et == 0), stop=False)
        nc.tensor.matmul(ps[:], W_sb[:, enct, :], HT[:], start=False, stop=True)
        GT = att.tile([c.OUTH, c.L], F32, tag=f"GT{half}")
        nc.scalar.activation(GT[:], ps[:], fn, bias=b_sb[:])
        for l in range(lt):
            pst = psum_s.tile([128, 128], F32, tag="ps_small")
            nc.tensor.transpose(pst[:], GT[:, l * 128:(l + 1) * 128], ident[:])
            nc.vector.tensor_copy(
                onat[:, l, half * c.OUTH:(half + 1) * c.OUTH], pst[:, :c.OUTH])
    for l in range(lt):
        nc.sync.dma_start(out_dram[l * 128:(l + 1) * 128, :], onat[:, l, :])

    for p in (att2, att, dram, psum_s, psum, state, const):
        p.release()


# --------------------------------------------------------------------------
# Host side
# --------------------------------------------------------------------------

def preprocess(inputs, c: Cfg):
    """Build per-core input maps from the full problem inputs (host, numpy).

    Only index-derived quantities (counts/degrees) and relayouts happen here;
    all float math on node/sequence data runs on device.
    """
    x = np.asarray(inputs["x"], np.float32)
    nf = np.asarray(inputs["node_features"], np.float32)
    ei = np.asarray(inputs["edge_index"])
    hi = np.asarray(inputs["hyperedge_index"])
    NP, NHE, BS = c.NP, c.NHE, c.BS
    N = BS * NP

    src, dst = np.asarray(ei[0], np.int64), np.asarray(ei[1], np.int64)
    pn, pe = np.asarray(hi[:, 0], np.int64), np.asarray(hi[:, 1], np.int64)

    dout = np.bincount(src, minlength=N).astype(np.float64)
    din = np.bincount(dst, minlength=N).astype(np.float64)
    rs_dout = (1.0 / np.sqrt(np.maximum(dout, 1.0))).astype(np.float32)
    rs_din = (1.0 / np.sqrt(np.maximum(din, 1.0))).astype(np.float32)
    D = np.bincount(pn, minlength=N).astype(np.float64)
    B = np.bincount(pe, minlength=NHE).astype(np.float64)
    Dinv = np.where(D > 0, 1.0 / np.maximum(D, 1), 0.0).astype(np.float32)
    Binv = np.where(B > 0, 1.0 / np.maximum(B, 1), 0.0).astype(np.float32)

    W = {k: np.asarray(inputs[k], np.float32) for k in
         ("Wg1", "bg1", "Wg2", "bg2", "Wh1", "bh1", "Wh2", "bh2",
          "Wm", "bm", "Wm2", "bm2", "Ws", "bs_", "Wt", "bt")}
    bgW1 = (W["bg1"] @ W["Wh1"]).astype(np.float32)
    bgW2 = (W["bg2"] @ W["Wh2"]).astype(np.float32)

    g_of_e = src // NP
    assert (g_of_e == dst // NP).all(), "edges must stay within graphs"
    g_of_p = pn // NP

    common = {
        "binv": np.ascontiguousarray(Binv.reshape(c.het, 128).T),
        "bgW1": bgW1.astype(NP_F16)[None, :],
        "bgW2": bgW2.astype(NP_F16)[None, :],
        "Wg1": W["Wg1"].astype(NP_F16), "Wg2": W["Wg2"].astype(NP_F16),
        "Wh1": W["Wh1"].astype(NP_F16), "Wh2": W["Wh2"].astype(NP_F16),
        "Wm": W["Wm"], "Wm2": W["Wm2"].astype(NP_F16),
        "Ws": W["Ws"], "Wt": W["Wt"],
        "bh1": W["bh1"].astype(np.float32)[:, None],
        "bh2": W["bh2"].astype(np.float32)[:, None],
        "bm": W["bm"].astype(np.float32)[:, None],
        "bm2": W["bm2"].astype(np.float32)[:, None],
        "bs": W["bs_"].astype(np.float32)[:, None],
        "bt": W["bt"].astype(np.float32)[:, None],
    }

    def regroup(M, cw, blk):
        # [R, C] -> [C//cw, R//(128*blk), 128, blk, cw]; per-partition bytes
        # of one streaming tile are contiguous in DRAM.
        R, C = M.shape
        return np.ascontiguousarray(
            M.reshape(R // (128 * blk), blk, 128, C // cw, cw)
            .transpose(3, 0, 2, 1, 4))

    in_maps = []
    for gidx in range(c.n_cores):
        lo = gidx * NP
        m = dict(common)
        m["nfrT"] = np.ascontiguousarray(nf[gidx].T).astype(NP_F16)
        m["xT"] = np.ascontiguousarray(x[gidx].T)
        m["rdo"] = np.ascontiguousarray(rs_dout[lo:lo + NP].reshape(c.nt, 128).T)
        m["rdi"] = np.ascontiguousarray(rs_din[lo:lo + NP].reshape(c.nt, 128).T)
        m["dinv_row"] = Dinv[lo:lo + NP].astype(NP_F16)[None, :]

        sel = g_of_e == gidx
        ss, dd = src[sel] - lo, dst[sel] - lo
        cnt = np.bincount(ss * NP + dd, minlength=NP * NP)
        assert cnt.max() <= 16, f"edge multiplicity {cnt.max()} > 16"
        AT = cnt.astype(NP_F8).reshape(NP, NP)          # [s, d]
        del cnt
        m["AT"] = regroup(AT, c.DCW, c.SBLK)
        del AT

        selp = g_of_p == gidx
        nn, ee = pn[selp] - lo, pe[selp]
        cnt = np.bincount(nn * NHE + ee, minlength=NP * NHE)
        assert cnt.max() <= 16, f"incidence multiplicity {cnt.max()} > 16"
        HN = cnt.astype(NP_F8).reshape(NP, NHE)          # [n, he]
        del cnt
        m["HN"] = regroup(HN, c.HECW, c.SBLK)
        m["HET"] = regroup(np.ascontiguousarray(HN.T), c.DCW, c.HBLK)
        del HN

        for name, shape, _, npdt in input_specs(c):
            arr = np.ascontiguousarray(m[name]).astype(npdt, copy=False)
            assert list(arr.shape) == list(shape), (name, arr.shape, shape)
            m[name] = arr
        in_maps.append(m)
    return in_maps


_PROGRAM_CACHE = {}


def _get_program(c: Cfg):
    k = c.key()
    if k not in _PROGRAM_CACHE:
        _PROGRAM_CACHE[k] = build_program(c)
    return _PROGRAM_CACHE[k]


def run(inputs, c: Cfg, trace=False):
    nc = _get_program(c)
    in_maps = preprocess(inputs, c)
    res = run_bass_kernel_spmd(nc, in_maps, list(range(c.n_cores)), trace=trace)
    out = np.stack([res.results[i]["out"] for i in range(c.n_cores)], axis=0)
    return out.astype(np.float32), res


def kernel(**inputs) -> np.ndarray:
    c = Cfg()
    out, _ = run(inputs, c)
    return out
